# revision 4
# baseline (speedup 1.0000x reference)
"""Structured butterfly kernel, bf16 datapath, software-pipelined.

x is cast to bf16 AND transposed per-shard on host, so the device input
loads are straight (full-bandwidth) batched DMAs — one per 1024-row super
— instead of 8 xbar-transpose DMAs.  Stages 0-6 (128x128 block-diag) run
as data-stationary bf16 matmuls; stages 7-9 as one PE transpose pass +
bf16 matmuls against 16x block-diag(8x8) weights.  Output is stored bf16
(halves store traffic) and upcast to fp32 on host.

The per-tile chain MM-A -> y-copy -> T -> zt-copy -> MM-B -> o-copy is
emitted software-pipelined (tile i's MM-A, tile i-1's T, tile i-2's MM-B
per step) so each PSUM-drain copy has a full tile-time of slack before
its PE consumer.  PSUM: mma 2x1 + tp2 2x1 + mmb 2x2 banks = 8.
"""

import numpy as np
import ml_dtypes

import concourse.bacc as bacc
import concourse.mybir as mybir
import concourse.tile as tile
from concourse.bass_utils import run_bass_kernel_spmd
from concourse.masks import make_identity

N_CORES = 8
BATCH = 32768
DIM = 1024
STAGES = 10
P = 128
ROWS_PER_CORE = BATCH // N_CORES          # 4096
R_SUPER = 1024                            # rows per batched DMA (in and out)
N_SUPER = ROWS_PER_CORE // R_SUPER        # 4
N_TILES = ROWS_PER_CORE // P              # 32
N_CHUNKS = DIM // P                       # 8
H4 = P * 4
F32 = mybir.dt.float32
BF16 = mybir.dt.bfloat16

_NC = {}


def _stage_product(angles: np.ndarray, stages) -> np.ndarray:
    B = np.eye(DIM, dtype=np.float64)
    k = np.arange(DIM)
    for s in stages:
        stride = 1 << s
        b = k // (2 * stride)
        j = k % stride
        h = (k >> s) & 1
        th = angles[s].astype(np.float64)[b * stride + j]
        C = np.cos(th)
        S = np.where(h == 0, -np.sin(th), np.sin(th))
        B = C[:, None] * B + S[:, None] * B[k ^ stride]
    return B


def _build_weights(angles: np.ndarray):
    """Returns (WA [128, 1024], WB [128, 1024]) bf16."""
    B_lo = _stage_product(angles, range(7))
    B_hi = _stage_product(angles, range(7, 10))

    WA = np.zeros((P, N_CHUNKS * P), dtype=np.float64)
    for h in range(N_CHUNKS):
        blk = B_lo[h * P:(h + 1) * P, h * P:(h + 1) * P]
        WA[:, h * P:(h + 1) * P] = blk.T

    jj = np.arange(P)
    H = np.zeros((P, 8, 8), dtype=np.float64)
    for hp in range(8):
        for h in range(8):
            H[:, hp, h] = B_hi[hp * P + jj, h * P + jj]

    WB = np.zeros((P, N_CHUNKS * P), dtype=np.float64)
    for g in range(8):
        blk = np.zeros((P, P), dtype=np.float64)
        for j16 in range(16):
            j = 16 * g + j16
            for h in range(8):
                for hp in range(8):
                    blk[h * 16 + j16, hp * 16 + j16] = H[j, hp, h]
        WB[:, g * P:(g + 1) * P] = blk
    return (WA.astype(ml_dtypes.bfloat16), WB.astype(ml_dtypes.bfloat16))


def _build_nc(repeat: int = 1):
    nc = bacc.Bacc(
        "TRN2", target_bir_lowering=False, debug=False, num_devices=N_CORES
    )
    # host-transposed shard: xt_in[i, r] = x[r, i] (bf16)
    xt_in = nc.dram_tensor(
        "xt", [DIM, ROWS_PER_CORE], BF16, kind="ExternalInput"
    ).ap()
    wa_in = nc.dram_tensor("wa", [P, DIM], BF16, kind="ExternalInput").ap()
    wb_in = nc.dram_tensor("wb", [P, DIM], BF16, kind="ExternalInput").ap()
    out = nc.dram_tensor(
        "out", [ROWS_PER_CORE, DIM], BF16, kind="ExternalOutput"
    ).ap()

    NT = N_TILES * repeat
    NS = NT // (R_SUPER // P)

    with tile.TileContext(nc) as tc:
        from contextlib import ExitStack

        with ExitStack() as ctx:
            const = ctx.enter_context(tc.tile_pool(name="const", bufs=1))
            ident = const.tile([P, P], BF16)
            make_identity(nc, ident)

            wa_sb = const.tile([P, DIM], BF16)
            nc.sync.dma_start(wa_sb[:], wa_in[:])
            wb_sb = const.tile([P, DIM], BF16)
            nc.sync.dma_start(wb_sb[:], wb_in[:])

            mma = ctx.enter_context(tc.tile_pool(name="mma", bufs=2, space="PSUM"))
            tp2 = ctx.enter_context(tc.tile_pool(name="tp2", bufs=2, space="PSUM"))
            mmb = ctx.enter_context(tc.tile_pool(name="mmb", bufs=2, space="PSUM"))

            # Consume identity on PE early (single-wait discipline).
            warm = tp2.tile([P, DIM], BF16, tag="pt2")
            nc.tensor.transpose(warm[:, 0:P], ident[:], ident[:])

            xt_pool = ctx.enter_context(tc.tile_pool(name="xt", bufs=2))
            y_pool = ctx.enter_context(tc.tile_pool(name="y", bufs=3))
            zt_pool = ctx.enter_context(tc.tile_pool(name="zt", bufs=3))
            o_pool = ctx.enter_context(tc.tile_pool(name="o", bufs=2))

            xt_tiles = {}
            y_tiles = {}
            zt_tiles = {}
            o_tiles = {}

            xt_src = xt_in.rearrange("(h p) r -> p h r", h=N_CHUNKS)

            def load_xt(s):
                t = xt_pool.tile([P, N_CHUNKS, R_SUPER], BF16, tag="xt")
                r0 = (s % N_SUPER) * R_SUPER
                nc.sync.dma_start(t[:], xt_src[:, :, r0:r0 + R_SUPER])
                xt_tiles[s] = t

            def mm_a(i):
                s, rr = i // 8, i % 8
                xt = xt_tiles[s]
                if rr == 7:
                    del xt_tiles[s]
                y_t = y_pool.tile([P, DIM], BF16, tag="y")
                y_scatter = y_t[:].rearrange("p (g h j) -> p h g j", g=8, h=8)
                banks = []
                for q in range(2):
                    bank = mma.tile([P, H4], F32, tag="pa")
                    for hh in range(4):
                        h = 4 * q + hh
                        nc.tensor.matmul(
                            bank[:, hh * P:(hh + 1) * P],
                            xt[:, h, rr * P:(rr + 1) * P],
                            wa_sb[:, h * P:(h + 1) * P],
                            start=True,
                            stop=True,
                        )
                    banks.append(bank)
                # copy split: q0 + first chunk of q1 -> DVE, rest -> Act
                b0 = banks[0][:].rearrange("p (h g j) -> p h g j", h=4, g=8)
                b1 = banks[1][:].rearrange("p (h g j) -> p h g j", h=4, g=8)
                nc.vector.tensor_copy(out=y_scatter[:, 0:4], in_=b0)
                nc.vector.tensor_copy(out=y_scatter[:, 4:5], in_=b1[:, 0:1])
                nc.scalar.copy(out=y_scatter[:, 5:8], in_=b1[:, 1:4])
                y_tiles[i] = y_t

            def t_pass(i):
                y_t = y_tiles.pop(i)
                bank_t2 = tp2.tile([P, DIM], BF16, tag="pt2")
                for g in range(8):
                    nc.tensor.transpose(
                        bank_t2[:, g * P:(g + 1) * P],
                        y_t[:, g * P:(g + 1) * P],
                        ident[:],
                    )
                zt_q = zt_pool.tile([P, DIM], BF16, tag="zt")
                nc.vector.tensor_copy(out=zt_q[:], in_=bank_t2[:])
                zt_tiles[i] = zt_q

            def mm_b(i):
                s, rr = i // 8, i % 8
                zt_q = zt_tiles.pop(i)
                if rr == 0:
                    o_sup = o_pool.tile([P, 8 * DIM], BF16, tag="o", name=f"o{s}")
                    o_tiles[s] = o_sup
                o_t = o_tiles[s]
                bank_b = mmb.tile([P, DIM], F32, tag="pb")
                for g in range(8):
                    nc.tensor.matmul(
                        bank_b[:, g * P:(g + 1) * P],
                        zt_q[:, g * P:(g + 1) * P],
                        wb_sb[:, g * P:(g + 1) * P],
                        start=True,
                        stop=True,
                    )
                o_scatter = o_t[:, rr * DIM:(rr + 1) * DIM].rearrange(
                    "p (h g j) -> p g h j", h=8, g=8
                )
                nc.scalar.copy(
                    out=o_scatter,
                    in_=bank_b[:].rearrange("p (g h j) -> p g h j", g=8, h=8),
                )
                if rr == 7:
                    o_done = o_tiles.pop(s)
                    r0 = (s % N_SUPER) * R_SUPER
                    dst = out[r0:r0 + R_SUPER, :].rearrange(
                        "(q p) c -> p q c", q=8
                    )
                    nc.sync.dma_start(
                        dst, o_done[:].rearrange("p (q c) -> p q c", q=8)
                    )

            load_xt(0)
            for i in range(NT + 2):
                if i < NT:
                    if i % 8 == 0 and i // 8 + 1 < NS:
                        load_xt(i // 8 + 1)
                    mm_a(i)
                if 0 <= i - 1 < NT:
                    t_pass(i - 1)
                if 0 <= i - 2 < NT:
                    mm_b(i - 2)

    nc.compile()
    return nc


def _get_nc(repeat: int = 1):
    if repeat not in _NC:
        _NC[repeat] = _build_nc(repeat)
    return _NC[repeat]


def prepare_in_maps(x, angles):
    WA, WB = _build_weights(angles)
    xb = np.asarray(x, dtype=np.float32).astype(ml_dtypes.bfloat16)
    shards = xb.reshape(N_CORES, ROWS_PER_CORE, DIM)
    return [
        {
            "xt": np.ascontiguousarray(shards[i].T),
            "wa": WA,
            "wb": WB,
        }
        for i in range(N_CORES)
    ]


def host_ref(x, angles):
    B = _stage_product(angles, range(STAGES))
    return x.astype(np.float64) @ B.T


def kernel(x: np.ndarray, angles: np.ndarray) -> np.ndarray:
    x = np.ascontiguousarray(np.asarray(x, dtype=np.float32))
    angles = np.asarray(angles, dtype=np.float32)
    assert x.shape == (BATCH, DIM), x.shape

    in_maps = prepare_in_maps(x, angles)

    nc = _get_nc()
    res = run_bass_kernel_spmd(nc, in_maps, list(range(N_CORES)))
    out = np.concatenate([res.results[i]["out"] for i in range(N_CORES)], axis=0)
    return out.astype(np.float32)
